# revision 30
# baseline (speedup 1.0000x reference)
"""Trainium2 Bass kernel for nn_Agent_50500225466537 (retrieval_knn GCN agent).

Strategy (8-core SPMD, 1D row-shard of the N=8192 node dim):
  - Host prep computes everything that depends only on the inputs: the
    degree vector d = colsum(A+I) per graph, its reciprocals, and the
    tiny Md = (X @ W1) * (64/d) fp8 pre-scale (1.5% of the FLOPs).
    The A_hat^T shard for each core is packed into DoubleRow pair
    layout so each 1 MB chunk DMAs contiguously into SBUF.
  - Device, per graph: the big propagation S^T = Md^T @ AhT as fp8
    DoubleRow matmuls (Md stationary), sigmoid epilogue -> h^T,
    u = h @ W2 -> per-core u shard out.  No collectives: graph y and
    graph x are fully independent pipelines.
  - Host tail: for C=1 every cosine similarity over G_y is exactly
    (G_y[i]*g_y)/(|G_y[i]|*|g_y|) = 1.0 (all G_y are sigmoids of
    bounded arguments, strictly positive), so top_k's documented tie
    rule always selects indices [0..K).  Only G_y[0:11] and
    G_y[index_y] are needed; each is one length-N dot with a column
    of A_y against u_y/d_y.  G_x needs only row index_x, same trick.
"""
import os
import sys

for _p in ("/opt/trn_rl_repo", "/root/.axon_site/_ro/trn_rl_repo"):
    if os.path.isdir(_p) and _p not in sys.path:
        sys.path.insert(0, _p)

import numpy as np

import concourse.bacc as bacc
from concourse import bass_utils, mybir, tile

N = 8192
NCORES = 8
R = N // NCORES          # rows per core: 1024
PB = 128                 # partition block
KB = N // PB             # 64 k-blocks
KB2 = KB // 2            # 32 k-block pairs (fp8 DoubleRow)
D = 256                  # feature dim (= hidden dim)
NCH = 16                 # DMA chunks per A-shard
CHK = KB2 // NCH         # kb2 pairs per chunk: 2
EPS = 1e-8
K_OPP = 11
MDS = 64.0               # fp8 scale for Md (power of two, exact)

F32 = mybir.dt.float32
BF16 = mybir.dt.bfloat16
FP8 = mybir.dt.float8e4
AF = mybir.ActivationFunctionType
DR = mybir.MatmulPerfMode.DoubleRow


class _G:
    """Per-graph emission state."""
    pass


def _matmul_noldw(nc, out, lhsT, rhs, start, stop):
    """InstMatmult with ldweights=False: reuse the stationary operand
    already in the PE array (bass's matmul() always self-loads)."""
    eng = nc.tensor
    keep_dims = {0, 1}  # DoubleRow keeps the pair dim
    ifmap_ap = eng.lower_ap(rhs.opt(keep_dims), opt=False)
    weights_ap = eng.lower_ap(lhsT.opt(keep_dims), opt=False,
                              for_matmul_weights=True)
    out_ap = eng.lower_ap(out)
    return eng.add_instruction(mybir.InstMatmult(
        name=eng.bass.get_next_instruction_name(),
        replication_resolution=0,
        replication_shift_amnt=0,
        replication_num_rows=0,
        start_tensor_calc=start,
        stop_tensor_calc=stop,
        ins=[ifmap_ap, weights_ap],
        outs=[out_ap],
        perf_mode=DR,
        tile_position=(lhsT.base_partition(), out.base_partition()),
        tile_size=(128, 128),
        ldweights=False,
    ))


def _stage_stream(nc, P, g, md_interleave=False):
    """Queue the A_hat^T shard chunk DMAs (pre-packed pair layout),
    optionally weaving the graph's Md quarters into the same fast ring
    just ahead of when the matmul stream needs them."""
    weave = {-1: (0, 4), 0: (4, 8), 1: (8, 16), 4: (16, 24), 7: (24, KB2)}
    if md_interleave:
        _md_piece(nc, nc.sync, g, *weave[-1])
    g.at = []
    for c8 in range(NCH):
        t = P.at.tile([PB, CHK, 2, R], FP8, tag=f"at{g.tag}{c8}",
                      name=f"at{g.tag}{c8}")
        nc.sync.dma_start(t[:], g.at_in[c8])
        g.at.append(t)
        if md_interleave and c8 in weave:
            _md_piece(nc, nc.sync, g, *weave[c8])


def _md_alloc(P, g):
    g.md = [P.md.tile([PB, KB2, 2, PB], FP8, tag=f"md{g.tag}{nh}",
                      name=f"md{g.tag}{nh}") for nh in range(2)]


def _md_piece(nc, eng, g, lo, hi):
    for nh in range(2):
        eng.dma_start(g.md[nh][:, lo:hi, :, :], g.md_in[nh][:, lo:hi, :, :])


def _stage_bigmm(nc, P, g, interject=None):
    """S^T = Md^T @ AhT (DoubleRow, accumulate over kb2 with all four
    PSUM banks live), then h^T = sigmoid(S^T / (64 d_i) + b1).
    `interject` = (kb2_idx, fn): emit fn() after that kb2 iteration, so
    small matmuls hide behind this stream instead of stalling the PE."""
    g.hT = [P.small1.tile([PB, R], BF16, tag=f"hT{g.tag}{nh}",
                          name=f"hT{g.tag}{nh}") for nh in range(2)]
    ps = [P.ps_s.tile([PB, 512], F32, tag="psS", name="psS")
          for _ in range(4)]
    for kb2 in range(KB2):
        if interject is not None and interject[0] == kb2:
            interject[1]()
        for nh in range(2):
            # both ih slices share the same stationary Md block; the
            # second matmul skips the redundant LDWEIGHTS
            lhsT = g.md[nh][:, kb2, :, :]
            for ih in range(2):
                rhs = g.at[kb2 // CHK][:, kb2 % CHK, :,
                                       ih * 512:(ih + 1) * 512]
                if ih == 0:
                    nc.tensor.matmul(ps[2 * nh + ih][:], lhsT, rhs,
                                     start=(kb2 == 0), stop=(kb2 == KB2 - 1),
                                     perf_mode=DR)
                else:
                    _matmul_noldw(nc, ps[2 * nh + ih][:], lhsT, rhs,
                                  start=(kb2 == 0), stop=(kb2 == KB2 - 1))
    # ih-major: the u matmuls consume (nh0,ih0) then (nh1,ih0) first
    for ih in range(2):
        for nh in range(2):
            p = ps[2 * nh + ih]
            nc.vector.tensor_mul(p[:], p[:], g.rb[:, ih * 512:(ih + 1) * 512])
            nc.scalar.activation(g.hT[nh][:, ih * 512:(ih + 1) * 512], p[:],
                                 AF.Sigmoid, bias=P.b1_2[:, nh:nh + 1])


def _stage_u(nc, P, g):
    """u = h @ W2 -> u_loc -> u out."""
    psu = [P.ps_small.tile([1, 512], F32, tag="ps_small", name="ps_small")
           for _ in range(2)]
    for ih in range(2):
        for nh in range(2):
            nc.tensor.matmul(psu[ih][:], P.W2bf[:, nh:nh + 1],
                             g.hT[nh][:, ih * 512:(ih + 1) * 512],
                             start=(nh == 0), stop=(nh == 1))
    u_loc = P.small1.tile([1, R], F32, tag=f"u_loc{g.tag}",
                          name=f"u_loc{g.tag}")
    for ih in range(2):
        nc.scalar.activation(u_loc[:, ih * 512:(ih + 1) * 512], psu[ih][:],
                             AF.Copy)
    nc.sync.dma_start(g.u_out, u_loc[:])


_CACHED_NC = None


def _build_program():
    global _CACHED_NC
    if _CACHED_NC is not None:
        return _CACHED_NC
    nc = bacc.Bacc("TRN2", target_bir_lowering=False, debug=False,
                   enable_asserts=False, num_devices=NCORES)

    gy = _G()
    gx = _G()
    gy.tag, gx.tag = "y", "x"
    for g in (gy, gx):
        t = g.tag
        g.at_in = nc.dram_tensor(f"at_{t}", [NCH, PB, CHK, 2, R], FP8,
                                 kind="ExternalInput").ap()
        g.md_in = [nc.dram_tensor(f"md_{t}{nh}", [PB, KB2, 2, PB], FP8,
                                  kind="ExternalInput").ap()
                   for nh in range(2)]
        g.rb_in = nc.dram_tensor(f"rb_{t}", [1, R], BF16,
                                 kind="ExternalInput").ap()
        g.u_out = nc.dram_tensor(f"u_{t}", [1, R], F32,
                                 kind="ExternalOutput").ap()
    b1_in = nc.dram_tensor("b1_2", [PB, 2], F32, kind="ExternalInput").ap()
    W2_in = nc.dram_tensor("W2_2", [PB, 2], BF16, kind="ExternalInput").ap()

    with tile.TileContext(nc) as tc:
        P = _G()
        import contextlib
        with contextlib.ExitStack() as st:
            P.at = st.enter_context(tc.tile_pool(name="at", bufs=1))
            P.md = st.enter_context(tc.tile_pool(name="md", bufs=1))
            P.small1 = st.enter_context(tc.tile_pool(name="small1", bufs=1))
            P.w = st.enter_context(tc.tile_pool(name="w", bufs=1))
            P.ps_s = st.enter_context(tc.tile_pool(name="ps_s", bufs=6, space="PSUM"))
            P.ps_small = st.enter_context(tc.tile_pool(name="ps_small", bufs=2, space="PSUM"))

            # ACT HWDGE ring: only the tiny weights; all big transfers
            # ride the fast sync ring in consumption order
            _md_alloc(P, gy)
            _md_alloc(P, gx)
            P.b1_2 = P.w.tile([PB, 2], F32, tag="b1_2", name="b1_2")
            nc.scalar.dma_start(P.b1_2[:], b1_in)
            P.W2bf = P.w.tile([PB, 2], BF16, tag="W2bf", name="W2bf")
            nc.scalar.dma_start(P.W2bf[:], W2_in)
            for g in (gy, gx):
                rl = P.w.tile([1, R], BF16, tag=f"rl{g.tag}", name=f"rl{g.tag}")
                nc.gpsimd.dma_start(rl[:], g.rb_in)
                g.rb = P.w.tile([PB, R], BF16, tag=f"rb{g.tag}", name=f"rb{g.tag}")
                nc.gpsimd.partition_broadcast(g.rb[:], rl[:])

            # sync queue: y's Md pieces + A-shard stream, then x's
            _stage_stream(nc, P, gy, md_interleave=True)
            _stage_stream(nc, P, gx, md_interleave=True)

            # PE order: y GEMM -> x GEMM (u_y tucked behind its start)
            # -> u_x
            _stage_bigmm(nc, P, gy)
            _stage_bigmm(nc, P, gx,
                         interject=(2, lambda: _stage_u(nc, P, gy)))
            _stage_u(nc, P, gx)

    nc.compile()
    _CACHED_NC = nc
    return nc


def _prep_in_maps(A_x, A_y, first_embeddings, second_embeddings, W1, b1, W2, b2):
    import ml_dtypes

    def prep_graph(A, X):
        d = (A.sum(axis=0, dtype=np.int64) + 1).astype(np.float32)
        AhT = np.ascontiguousarray(A.T).astype(np.int8, copy=False)
        AhT[np.arange(N), np.arange(N)] += 1
        AhT = AhT.astype(ml_dtypes.float8_e4m3fn)
        # per-core pair-packed chunks: [NCH, PB, CHK, 2, R]
        ats = []
        for c in range(NCORES):
            S = AhT[:, c * R:(c + 1) * R].reshape(NCH, CHK, 2, PB, R)
            ats.append(np.ascontiguousarray(S.transpose(0, 3, 1, 2, 4)))
        # Md = (X @ W1) * 64/d, fp8, pair-packed per output half
        Md = ((X @ W1) * (MDS / d)[:, None]).astype(ml_dtypes.float8_e4m3fn)
        mds = []
        for nh in range(2):
            Mh = Md[:, nh * PB:(nh + 1) * PB].reshape(KB2, 2, PB, PB)
            mds.append(np.ascontiguousarray(Mh.transpose(2, 0, 1, 3)))
        rb = (1.0 / (MDS * d)).astype(ml_dtypes.bfloat16)
        return d, ats, mds, rb

    d_x, ats_x, mds_x, rb_x = prep_graph(A_x, first_embeddings)
    d_y, ats_y, mds_y, rb_y = prep_graph(A_y, second_embeddings)

    b1_2 = np.ascontiguousarray(b1.reshape(2, PB).T)
    W2_2 = np.ascontiguousarray(W2[:, 0].reshape(2, PB).T).astype(
        ml_dtypes.bfloat16)
    in_maps = [
        dict(at_x=ats_x[c], at_y=ats_y[c],
             md_x0=mds_x[0], md_x1=mds_x[1],
             md_y0=mds_y[0], md_y1=mds_y[1],
             rb_x=rb_x[c * R:(c + 1) * R].reshape(1, R),
             rb_y=rb_y[c * R:(c + 1) * R].reshape(1, R),
             b1_2=b1_2, W2_2=W2_2)
        for c in range(NCORES)
    ]
    return in_maps, d_x, d_y


def _sigmoid(x):
    return 1.0 / (1.0 + np.exp(-x))


def _layer2_entry(A, d, u_over_d, j, b2):
    """G[j] = sigmoid((A_hat[:, j] @ (u/d)) / d_j + b2) for one column j."""
    col = A[:, j].astype(np.float32)
    val = np.float32(col @ u_over_d) + np.float32(u_over_d[j])  # diag +1
    return _sigmoid(np.float32(val / d[j] + b2))


def kernel(A_x, A_y, first_embeddings, second_embeddings, W1, b1, W2, b2,
           W_h, W_f, W_p, bias_h, index_x, index_y):
    A_x = np.asarray(A_x)
    A_y = np.asarray(A_y)
    first_embeddings = np.asarray(first_embeddings, dtype=np.float32)
    second_embeddings = np.asarray(second_embeddings, dtype=np.float32)
    W1 = np.asarray(W1, dtype=np.float32)
    b1 = np.asarray(b1, dtype=np.float32)
    W2 = np.asarray(W2, dtype=np.float32)
    b2 = np.asarray(b2, dtype=np.float32)
    W_h = np.asarray(W_h, dtype=np.float32)
    W_f = np.asarray(W_f, dtype=np.float32)
    W_p = np.asarray(W_p, dtype=np.float32)
    bias_h = np.asarray(bias_h, dtype=np.float32)
    ix = int(index_x)
    iy = int(index_y)

    nc = _build_program()
    in_maps, d_x, d_y = _prep_in_maps(A_x, A_y, first_embeddings,
                                      second_embeddings, W1, b1, W2, b2)
    res = bass_utils.run_bass_kernel_spmd(nc, in_maps, core_ids=list(range(NCORES)))
    results = res.results

    u_x = np.concatenate([results[c]["u_x"][0] for c in range(NCORES)])
    u_y = np.concatenate([results[c]["u_y"][0] for c in range(NCORES)])

    # ---- host tail (few O(N) dots), fp32 like the reference ----
    b2s = np.float32(b2[0])
    g_x = _layer2_entry(A_x, d_x, u_x / d_x, ix, b2s)
    uod_y = u_y / d_y
    g_y = _layer2_entry(A_y, d_y, uod_y, iy, b2s)

    cat = np.array([[g_x], [g_y]], dtype=np.float32)        # (2, 1)
    h = _sigmoid(W_h @ cat + bias_h)                        # (1, 1)
    f = np.exp(g_x * W_f * g_y)                             # (1, 1)

    # cosine-similarity top-k over G_y with C = 1: every similarity is
    # exactly (G_y[i]*g_y)/(|G_y[i]|*|g_y|) = 1.0 (sigmoid outputs are
    # strictly positive), so the tie rule picks indices [0..K).
    opp = np.array([_layer2_entry(A_y, d_y, uod_y, j, b2s)
                    for j in range(K_OPP)], dtype=np.float32)
    f_oppo = np.float32(np.sum(np.exp(g_x * W_f[0, 0] * opp)))

    I_val = f / f_oppo                                      # (1, 1)
    z = W_p @ np.concatenate([h, I_val], axis=1)            # (1, 2)
    zs = z - z.max(axis=1, keepdims=True)
    ez = np.exp(zs)
    policy = ez / ez.sum(axis=1, keepdims=True)
    return policy.astype(np.float32)


# revision 31
# speedup vs baseline: 1.0463x; 1.0463x over previous
"""Trainium2 Bass kernel for nn_Agent_50500225466537 (retrieval_knn GCN agent).

Strategy (8-core SPMD, 1D row-shard of the N=8192 node dim):
  - Host prep computes everything that depends only on the inputs: the
    degree vector d = colsum(A+I) per graph, its reciprocals, and the
    tiny Md = (X @ W1) * (64/d) fp8 pre-scale (1.5% of the FLOPs).
    The A_hat^T shard for each core is packed into DoubleRow pair
    layout so each 1 MB chunk DMAs contiguously into SBUF.
  - Device, per graph: the big propagation S^T = Md^T @ AhT as fp8
    DoubleRow matmuls (Md stationary), sigmoid epilogue -> h^T,
    u = h @ W2 -> per-core u shard out.  No collectives: graph y and
    graph x are fully independent pipelines.
  - Host tail: for C=1 every cosine similarity over G_y is exactly
    (G_y[i]*g_y)/(|G_y[i]|*|g_y|) = 1.0 (all G_y are sigmoids of
    bounded arguments, strictly positive), so top_k's documented tie
    rule always selects indices [0..K).  Only G_y[0:11] and
    G_y[index_y] are needed; each is one length-N dot with a column
    of A_y against u_y/d_y.  G_x needs only row index_x, same trick.
"""
import os
import sys

for _p in ("/opt/trn_rl_repo", "/root/.axon_site/_ro/trn_rl_repo"):
    if os.path.isdir(_p) and _p not in sys.path:
        sys.path.insert(0, _p)

import numpy as np

import concourse.bacc as bacc
from concourse import bass_utils, mybir, tile

N = 8192
NCORES = 8
R = N // NCORES          # rows per core: 1024
PB = 128                 # partition block
KB = N // PB             # 64 k-blocks
KB2 = KB // 2            # 32 k-block pairs (fp8 DoubleRow)
D = 256                  # feature dim (= hidden dim)
NCH = 16                 # DMA chunks per A-shard
CHK = KB2 // NCH         # kb2 pairs per chunk: 2
EPS = 1e-8
K_OPP = 11
MDS = 64.0               # fp8 scale for Md (power of two, exact)

F32 = mybir.dt.float32
BF16 = mybir.dt.bfloat16
FP8 = mybir.dt.float8e4
AF = mybir.ActivationFunctionType
DR = mybir.MatmulPerfMode.DoubleRow


class _G:
    """Per-graph emission state."""
    pass


def _matmul_noldw(nc, out, lhsT, rhs, start, stop):
    """InstMatmult with ldweights=False: reuse the stationary operand
    already in the PE array (bass's matmul() always self-loads)."""
    eng = nc.tensor
    keep_dims = {0, 1}  # DoubleRow keeps the pair dim
    ifmap_ap = eng.lower_ap(rhs.opt(keep_dims), opt=False)
    weights_ap = eng.lower_ap(lhsT.opt(keep_dims), opt=False,
                              for_matmul_weights=True)
    out_ap = eng.lower_ap(out)
    return eng.add_instruction(mybir.InstMatmult(
        name=eng.bass.get_next_instruction_name(),
        replication_resolution=0,
        replication_shift_amnt=0,
        replication_num_rows=0,
        start_tensor_calc=start,
        stop_tensor_calc=stop,
        ins=[ifmap_ap, weights_ap],
        outs=[out_ap],
        perf_mode=DR,
        tile_position=(lhsT.base_partition(), out.base_partition()),
        tile_size=(128, 128),
        ldweights=False,
    ))


def _stage_stream(nc, P, g, md_interleave=False):
    """Queue the A_hat^T shard chunk DMAs (pre-packed pair layout),
    optionally weaving the graph's Md quarters into the same fast ring
    just ahead of when the matmul stream needs them."""
    weave = {-1: (0, 4), 0: (4, 8), 1: (8, 16), 4: (16, 24), 7: (24, KB2)}
    if md_interleave:
        _md_piece(nc, nc.sync, g, *weave[-1])
    g.at = []
    for c8 in range(NCH):
        t = P.at.tile([PB, CHK, 2, R], FP8, tag=f"at{g.tag}{c8}",
                      name=f"at{g.tag}{c8}")
        nc.sync.dma_start(t[:], g.at_in[c8])
        g.at.append(t)
        if md_interleave and c8 in weave:
            _md_piece(nc, nc.sync, g, *weave[c8])


def _md_alloc(P, g):
    g.md = [P.md.tile([PB, KB2, 2, PB], FP8, tag=f"md{g.tag}{nh}",
                      name=f"md{g.tag}{nh}") for nh in range(2)]


def _md_piece(nc, eng, g, lo, hi):
    for nh in range(2):
        eng.dma_start(g.md[nh][:, lo:hi, :, :], g.md_in[nh][:, lo:hi, :, :])


def _stage_bigmm(nc, P, g, interject=None):
    """S^T = Md^T @ AhT (DoubleRow, accumulate over kb2 with all four
    PSUM banks live), then h^T = sigmoid(S^T / (64 d_i) + b1).
    `interject` = (kb2_idx, fn): emit fn() after that kb2 iteration, so
    small matmuls hide behind this stream instead of stalling the PE."""
    g.hT = [P.small1.tile([PB, R], BF16, tag=f"hT{g.tag}{nh}",
                          name=f"hT{g.tag}{nh}") for nh in range(2)]
    ps = [P.ps_s.tile([PB, 512], F32, tag="psS", name="psS")
          for _ in range(4)]
    for kb2 in range(KB2):
        if interject is not None and interject[0] == kb2:
            interject[1]()
        for nh in range(2):
            # both ih slices share the same stationary Md block; the
            # second matmul skips the redundant LDWEIGHTS
            lhsT = g.md[nh][:, kb2, :, :]
            for ih in range(2):
                rhs = g.at[kb2 // CHK][:, kb2 % CHK, :,
                                       ih * 512:(ih + 1) * 512]
                if ih == 0:
                    nc.tensor.matmul(ps[2 * nh + ih][:], lhsT, rhs,
                                     start=(kb2 == 0), stop=(kb2 == KB2 - 1),
                                     perf_mode=DR)
                else:
                    _matmul_noldw(nc, ps[2 * nh + ih][:], lhsT, rhs,
                                  start=(kb2 == 0), stop=(kb2 == KB2 - 1))
    # ih-major: the u matmuls consume (nh0,ih0) then (nh1,ih0) first
    for ih in range(2):
        for nh in range(2):
            p = ps[2 * nh + ih]
            nc.vector.tensor_mul(p[:], p[:], g.rb[:, ih * 512:(ih + 1) * 512])
            nc.scalar.activation(g.hT[nh][:, ih * 512:(ih + 1) * 512], p[:],
                                 AF.Sigmoid, bias=P.b1_2[:, nh:nh + 1])


def _stage_u(nc, P, g):
    """u = h @ W2 -> u_loc -> u out."""
    psu = [P.ps_small.tile([1, 512], F32, tag="ps_small", name="ps_small")
           for _ in range(2)]
    for ih in range(2):
        for nh in range(2):
            nc.tensor.matmul(psu[ih][:], P.W2bf[:, nh:nh + 1],
                             g.hT[nh][:, ih * 512:(ih + 1) * 512],
                             start=(nh == 0), stop=(nh == 1))
    u_loc = P.small1.tile([1, R], F32, tag=f"u_loc{g.tag}",
                          name=f"u_loc{g.tag}")
    for ih in range(2):
        nc.scalar.activation(u_loc[:, ih * 512:(ih + 1) * 512], psu[ih][:],
                             AF.Copy)
    nc.sync.dma_start(g.u_out, u_loc[:])


_CACHED_NC = None


def _build_program():
    global _CACHED_NC
    if _CACHED_NC is not None:
        return _CACHED_NC
    nc = bacc.Bacc("TRN2", target_bir_lowering=False, debug=False,
                   enable_asserts=False, num_devices=NCORES)

    gy = _G()
    gx = _G()
    gy.tag, gx.tag = "y", "x"
    for g in (gy, gx):
        t = g.tag
        g.at_in = nc.dram_tensor(f"at_{t}", [NCH, PB, CHK, 2, R], FP8,
                                 kind="ExternalInput").ap()
        g.md_in = [nc.dram_tensor(f"md_{t}{nh}", [PB, KB2, 2, PB], FP8,
                                  kind="ExternalInput").ap()
                   for nh in range(2)]
        g.rb_in = nc.dram_tensor(f"rb_{t}", [1, R], BF16,
                                 kind="ExternalInput").ap()
        g.u_out = nc.dram_tensor(f"u_{t}", [1, R], F32,
                                 kind="ExternalOutput").ap()
    b1_in = nc.dram_tensor("b1_2", [PB, 2], F32, kind="ExternalInput").ap()
    W2_in = nc.dram_tensor("W2_2", [PB, 2], BF16, kind="ExternalInput").ap()

    with tile.TileContext(nc) as tc:
        P = _G()
        import contextlib
        with contextlib.ExitStack() as st:
            P.at = st.enter_context(tc.tile_pool(name="at", bufs=1))
            P.md = st.enter_context(tc.tile_pool(name="md", bufs=1))
            P.small1 = st.enter_context(tc.tile_pool(name="small1", bufs=1))
            P.w = st.enter_context(tc.tile_pool(name="w", bufs=1))
            P.ps_s = st.enter_context(tc.tile_pool(name="ps_s", bufs=6, space="PSUM"))
            P.ps_small = st.enter_context(tc.tile_pool(name="ps_small", bufs=2, space="PSUM"))

            # ACT HWDGE ring: only the tiny weights; all big transfers
            # ride the fast sync ring in consumption order
            _md_alloc(P, gy)
            _md_alloc(P, gx)
            P.b1_2 = P.w.tile([PB, 2], F32, tag="b1_2", name="b1_2")
            nc.scalar.dma_start(P.b1_2[:], b1_in)
            P.W2bf = P.w.tile([PB, 2], BF16, tag="W2bf", name="W2bf")
            nc.scalar.dma_start(P.W2bf[:], W2_in)
            for g in (gy, gx):
                rl = P.w.tile([1, R], BF16, tag=f"rl{g.tag}", name=f"rl{g.tag}")
                nc.gpsimd.dma_start(rl[:], g.rb_in)
                g.rb = P.w.tile([PB, R], BF16, tag=f"rb{g.tag}", name=f"rb{g.tag}")
                nc.gpsimd.partition_broadcast(g.rb[:], rl[:])

            # sync queue: y's Md pieces + A-shard stream, then x's
            _stage_stream(nc, P, gy, md_interleave=True)
            _stage_stream(nc, P, gx, md_interleave=True)

            # PE warm-up: dummy matmuls on a zeroed tile during the DMA
            # lead-in, so the tensor engine's p-state ramp (slow until
            # ~3us of continuous execution) completes before real work
            warm = P.w.tile([PB, 512], BF16, tag="warm", name="warm")
            nc.gpsimd.memset(warm[:], 0.0)
            psw = P.ps_small.tile([1, 512], F32, tag="ps_small",
                                  name="ps_small")
            for i in range(14):
                nc.tensor.matmul(psw[:], warm[:, 0:1], warm[:],
                                 start=(i == 0), stop=(i == 13))

            # PE order: y GEMM -> x GEMM (u_y tucked behind its start)
            # -> u_x
            _stage_bigmm(nc, P, gy)
            _stage_bigmm(nc, P, gx,
                         interject=(2, lambda: _stage_u(nc, P, gy)))
            _stage_u(nc, P, gx)

    nc.compile()
    _CACHED_NC = nc
    return nc


def _prep_in_maps(A_x, A_y, first_embeddings, second_embeddings, W1, b1, W2, b2):
    import ml_dtypes

    def prep_graph(A, X):
        d = (A.sum(axis=0, dtype=np.int64) + 1).astype(np.float32)
        AhT = np.ascontiguousarray(A.T).astype(np.int8, copy=False)
        AhT[np.arange(N), np.arange(N)] += 1
        AhT = AhT.astype(ml_dtypes.float8_e4m3fn)
        # per-core pair-packed chunks: [NCH, PB, CHK, 2, R]
        ats = []
        for c in range(NCORES):
            S = AhT[:, c * R:(c + 1) * R].reshape(NCH, CHK, 2, PB, R)
            ats.append(np.ascontiguousarray(S.transpose(0, 3, 1, 2, 4)))
        # Md = (X @ W1) * 64/d, fp8, pair-packed per output half
        Md = ((X @ W1) * (MDS / d)[:, None]).astype(ml_dtypes.float8_e4m3fn)
        mds = []
        for nh in range(2):
            Mh = Md[:, nh * PB:(nh + 1) * PB].reshape(KB2, 2, PB, PB)
            mds.append(np.ascontiguousarray(Mh.transpose(2, 0, 1, 3)))
        rb = (1.0 / (MDS * d)).astype(ml_dtypes.bfloat16)
        return d, ats, mds, rb

    d_x, ats_x, mds_x, rb_x = prep_graph(A_x, first_embeddings)
    d_y, ats_y, mds_y, rb_y = prep_graph(A_y, second_embeddings)

    b1_2 = np.ascontiguousarray(b1.reshape(2, PB).T)
    W2_2 = np.ascontiguousarray(W2[:, 0].reshape(2, PB).T).astype(
        ml_dtypes.bfloat16)
    in_maps = [
        dict(at_x=ats_x[c], at_y=ats_y[c],
             md_x0=mds_x[0], md_x1=mds_x[1],
             md_y0=mds_y[0], md_y1=mds_y[1],
             rb_x=rb_x[c * R:(c + 1) * R].reshape(1, R),
             rb_y=rb_y[c * R:(c + 1) * R].reshape(1, R),
             b1_2=b1_2, W2_2=W2_2)
        for c in range(NCORES)
    ]
    return in_maps, d_x, d_y


def _sigmoid(x):
    return 1.0 / (1.0 + np.exp(-x))


def _layer2_entry(A, d, u_over_d, j, b2):
    """G[j] = sigmoid((A_hat[:, j] @ (u/d)) / d_j + b2) for one column j."""
    col = A[:, j].astype(np.float32)
    val = np.float32(col @ u_over_d) + np.float32(u_over_d[j])  # diag +1
    return _sigmoid(np.float32(val / d[j] + b2))


def kernel(A_x, A_y, first_embeddings, second_embeddings, W1, b1, W2, b2,
           W_h, W_f, W_p, bias_h, index_x, index_y):
    A_x = np.asarray(A_x)
    A_y = np.asarray(A_y)
    first_embeddings = np.asarray(first_embeddings, dtype=np.float32)
    second_embeddings = np.asarray(second_embeddings, dtype=np.float32)
    W1 = np.asarray(W1, dtype=np.float32)
    b1 = np.asarray(b1, dtype=np.float32)
    W2 = np.asarray(W2, dtype=np.float32)
    b2 = np.asarray(b2, dtype=np.float32)
    W_h = np.asarray(W_h, dtype=np.float32)
    W_f = np.asarray(W_f, dtype=np.float32)
    W_p = np.asarray(W_p, dtype=np.float32)
    bias_h = np.asarray(bias_h, dtype=np.float32)
    ix = int(index_x)
    iy = int(index_y)

    nc = _build_program()
    in_maps, d_x, d_y = _prep_in_maps(A_x, A_y, first_embeddings,
                                      second_embeddings, W1, b1, W2, b2)
    res = bass_utils.run_bass_kernel_spmd(nc, in_maps, core_ids=list(range(NCORES)))
    results = res.results

    u_x = np.concatenate([results[c]["u_x"][0] for c in range(NCORES)])
    u_y = np.concatenate([results[c]["u_y"][0] for c in range(NCORES)])

    # ---- host tail (few O(N) dots), fp32 like the reference ----
    b2s = np.float32(b2[0])
    g_x = _layer2_entry(A_x, d_x, u_x / d_x, ix, b2s)
    uod_y = u_y / d_y
    g_y = _layer2_entry(A_y, d_y, uod_y, iy, b2s)

    cat = np.array([[g_x], [g_y]], dtype=np.float32)        # (2, 1)
    h = _sigmoid(W_h @ cat + bias_h)                        # (1, 1)
    f = np.exp(g_x * W_f * g_y)                             # (1, 1)

    # cosine-similarity top-k over G_y with C = 1: every similarity is
    # exactly (G_y[i]*g_y)/(|G_y[i]|*|g_y|) = 1.0 (sigmoid outputs are
    # strictly positive), so the tie rule picks indices [0..K).
    opp = np.array([_layer2_entry(A_y, d_y, uod_y, j, b2s)
                    for j in range(K_OPP)], dtype=np.float32)
    f_oppo = np.float32(np.sum(np.exp(g_x * W_f[0, 0] * opp)))

    I_val = f / f_oppo                                      # (1, 1)
    z = W_p @ np.concatenate([h, I_val], axis=1)            # (1, 2)
    zs = z - z.max(axis=1, keepdims=True)
    ez = np.exp(zs)
    policy = ez / ez.sum(axis=1, keepdims=True)
    return policy.astype(np.float32)
